# revision 26
# baseline (speedup 1.0000x reference)
"""Bipartite NAND/NOR graph layer on 8 Trainium2 NeuronCores — Euler-stream
formulation.

Problem: out[i] = ~(x[a_i] & x[b_i]) if not nor_mask[i] else ~(x[a_i] | x[b_i])
with x: [32768, 2048] int32, (a, b): [32768, 2] indices, nor_mask: [32768] bool.

The baseline gathers TWO 2 KiB operand rows per output (3 rows of HBM traffic
per output incl. the write; ~96 MiB/core at the ~358 GB/s per-core HBM
roofline).  This kernel cuts traffic to ~2.7 rows/output and is paced by the
DVE select (see below):

- View outputs as edges of a multigraph over the 32768 input rows.  An Euler
  trail visits every edge once and consecutive trail edges SHARE a vertex, so
  gathering the trail's vertex sequence once gives both operands of every
  output: out[s] = f(stream[s], stream[s+1]).  Odd-degree vertices are paired
  with virtual edges (junk outputs the host discards); per-component trails
  are concatenated with one junk slot between components.  For this instance:
  32768 edges + 8052 junk slots -> ~1.25 gathered rows per output instead of 2.
- De Morgan on the complemented table cx = ~x: NAND = cx_a | cx_b,
  NOR = cx_a & cx_b.  The stream mixes both ops; the device computes AND and
  OR of each adjacent pair and resolves per-output with copy_predicated using
  a [128, dcol, 1] mask broadcast (stride 0) along the 512-word axis.  A
  2-instruction select is provably impossible with the available ALU ops, and
  bitwise TensorTensor is DVE-only (Pool rejects it), so the 3 DVE passes
  (~0.96 G col/s each) are the pacer: ~11.6 us per 1024-slot chunk vs ~11 us
  of DMA.  Measured ~306 us vs ~319 us for the 2-gather baseline in the same
  session.
- Chunk layout: a chunk gathers n slots as [128 partitions, ncol = n/128 slot
  columns]; idx order is permuted so slot (p, b) holds stream position
  base + (ncol-1)*p + b.  Each partition covers ncol-1 pairs; its last slot
  column duplicates the next partition's first slot (8/7 gather overhead), so
  pairs never cross partitions (compute engines cannot read partition-shifted
  views - BIR checkLegalPartitionAccess).  Small chunks at both ends of the
  schedule shorten pipeline ramp and drain.
- Rows are relabeled by first occurrence in the stream and the host uploads
  cx permuted accordingly, so the gather walks HBM nearly sequentially.
- Sharding: word axis split 4 ways (512-word = 2 KiB slices), stream split in
  2 halves -> 8 cores, zero cross-core traffic, one SPMD program (per-core
  data: x word-slice, half-stream index + mask tables).
- Ring layout per the baseline's findings: gathers alternate SWDGE rings 1/2,
  writes alone on ring 0 with single_packet=True, issued 2 chunks behind.
"""
import sys
sys.path.insert(0, "/opt/trn_rl_repo")

import numpy as np
from contextlib import ExitStack

import concourse.bass as bass
import concourse.tile as tile
from concourse import bacc, mybir
from concourse.bass import broadcast_tensor_aps
from concourse.bass_utils import run_bass_kernel_spmd

N_ROWS = 32768          # input rows == output rows
W_FULL = 2048           # int32 words per row
N_CORES = 8
WORD_SPLIT = 4
OUT_SPLIT = 2
WS = W_FULL // WORD_SPLIT   # 512 words per core slice (2 KiB)
P = 128
# The chunk PLAN is instance-derived in _prepare and shared by all 8 cores:
# a list of ('s', n) stream chunks (n gathered slots, (n/128-1)*128 pairs,
# 3-pass select) and ('c', n, op) classic chunks (n outputs, two gathers,
# single op pass).  Small stream chunks at the start shorten the ramp;
# classic chunks (DMA-heavy, DVE-light) are interleaved through the middle
# to balance engine load.
def _stream_sizes(pairs_needed):
    sizes = [512, 640]
    cap = 384 + 512
    while cap + 384 < pairs_needed:
        sizes.append(1024)
        cap += 896
    sizes.append(512)
    return sizes
CHUNK_QUEUES = (1, 2)   # gather ring by chunk parity
NUM_SWDGE_QUEUES = 3
WRITE_LAG = 2


def _euler_stream(output_node_input_indices, nor_mask, keep):
    """Build the Euler stream over the kept subset of the output multigraph.

    Returns (verts, edge_at, is_nand) where verts[s] is the input row gathered
    at stream slot s, edge_at[s] is the ORIGINAL output row computed from
    slots (s, s+1) (-1 for junk pairs), is_nand[s] its op.
    """
    mask = np.asarray(nor_mask).astype(bool)
    orig = np.flatnonzero(keep)
    idx = np.asarray(output_node_input_indices).astype(np.int64)[orig]
    m = idx.shape[0]
    n = N_ROWS

    deg = np.zeros(n, dtype=np.int64)
    np.add.at(deg, idx[:, 0], 1)
    np.add.at(deg, idx[:, 1], 1)

    # Components via union-find over real edges.
    parent = np.arange(n)

    def find(v):
        while parent[v] != v:
            parent[v] = parent[parent[v]]
            v = parent[v]
        return v

    for a, b in idx:
        ra, rb = find(a), find(b)
        if ra != rb:
            parent[ra] = rb
    root = np.array([find(v) for v in range(n)])

    comp_of = {}
    comps = []          # per component: list of member vertices with edges
    for v in range(n):
        if deg[v] == 0:
            continue
        r = root[v]
        c = comp_of.get(r)
        if c is None:
            c = comp_of[r] = len(comps)
            comps.append([])
        comps[c].append(v)

    # Virtual edges: within each component pair up odd vertices, leaving two
    # endpoints open (Euler path); fully even components get a circuit.
    starts = []
    extra = [[], []]    # endpoints of virtual edges
    for members in comps:
        odds = [v for v in members if deg[v] % 2 == 1]
        if odds:
            starts.append(odds[0])
            for i in range(1, len(odds) - 1, 2):
                extra[0].append(odds[i])
                extra[1].append(odds[i + 1])
        else:
            starts.append(members[0])

    # CSR adjacency over real + virtual edges.  Virtual edges have eid >= m.
    ea = np.concatenate([idx[:, 0], np.asarray(extra[0], dtype=np.int64)])
    eb = np.concatenate([idx[:, 1], np.asarray(extra[1], dtype=np.int64)])
    me = len(ea)
    half_v = np.concatenate([ea, eb])       # endpoint list, edge i at i, i+me
    half_o = np.concatenate([eb, ea])       # the other endpoint
    order = np.argsort(half_v, kind="stable")
    adj_other = half_o[order]
    adj_eid = (np.arange(2 * me) % me)[order]
    adj_start = np.zeros(n + 1, dtype=np.int64)
    np.add.at(adj_start, half_v + 1, 1)
    adj_start = np.cumsum(adj_start)

    used = np.zeros(me, dtype=bool)
    ptr = adj_start[:-1].copy()
    verts_l, edges_l = [], []
    for start in starts:
        # Iterative Hierholzer from `start`.
        stack_v = [start]
        stack_e = [-2]          # edge taken to REACH stack_v[i] (-2 for root)
        path_v, path_e = [], []
        while stack_v:
            v = stack_v[-1]
            pv = ptr[v]
            stop = adj_start[v + 1]
            while pv < stop and used[adj_eid[pv]]:
                pv += 1
            ptr[v] = pv
            if pv < stop:
                e = adj_eid[pv]
                used[e] = True
                ptr[v] = pv + 1
                stack_v.append(adj_other[pv])
                stack_e.append(e)
            else:
                path_v.append(stack_v.pop())
                path_e.append(stack_e.pop())
        path_v.reverse()
        path_e.reverse()
        # path_e[i+1] is the edge between path_v[i] and path_v[i+1].
        if verts_l:
            edges_l.append(-1)  # junk pair between components
        verts_l.extend(path_v)
        edges_l.extend(e if e < m else -1 for e in path_e[1:])

    verts = np.asarray(verts_l, dtype=np.int64)
    edge_at = np.asarray(edges_l, dtype=np.int64)
    assert len(verts) == len(edge_at) + 1
    assert np.count_nonzero(edge_at >= 0) == m
    real = edge_at >= 0
    edge_at[real] = orig[edge_at[real]]   # back to original output rows
    is_nand = np.zeros(len(edge_at), dtype=bool)
    is_nand[real] = ~mask[edge_at[real]]
    return verts, edge_at, is_nand


def _wrap_idxs(idx_chunk):
    """[n] int -> [128, n/16] int16 wrapped in 16 partitions, replicated
    across the 8 gpsimd core windows."""
    n = len(idx_chunk)
    assert n % 16 == 0
    blk = idx_chunk.astype(np.int16).reshape(n // 16, 16).T  # [16, n/16]
    return np.tile(blk, (8, 1))


def _prepare(output_node_input_indices, nor_mask):
    """Returns (relabel, idx_tabs, msk_tabs, row_orders, plan)."""
    idx = np.asarray(output_node_input_indices).astype(np.int64)
    mask = np.asarray(nor_mask).astype(bool)

    # Pull out a greedy matching of edges joining two odd-degree vertices:
    # removing such an edge kills one virtual (junk) trail pairing AND the
    # output it represents is computed in an op-pure classic 2-gather chunk
    # (1 DVE pass instead of 3).
    deg = np.zeros(N_ROWS, dtype=np.int64)
    np.add.at(deg, idx[:, 0], 1)
    np.add.at(deg, idx[:, 1], 1)
    odd = deg % 2 == 1
    taken = np.zeros(N_ROWS, dtype=bool)
    keep = np.ones(len(idx), dtype=bool)
    for e, (u, w) in enumerate(idx):
        if u != w and odd[u] and odd[w] and not taken[u] and not taken[w]:
            keep[e] = False
            taken[u] = taken[w] = True
    # Second round: pair remaining odd vertices via edge-disjoint 2-paths
    # (u-x, x-w): interior x keeps parity, u and w become even.  DMA-neutral
    # and saves 28 DVE column-units per pair.
    res = odd & ~taken          # still-odd, unresolved
    vadj = [[] for _ in range(N_ROWS)]
    for e, (u, w) in enumerate(idx):
        if keep[e] and u != w:
            vadj[u].append((e, w))
            vadj[w].append((e, u))
    eused = np.zeros(len(idx), dtype=bool)
    for x in range(N_ROWS):
        pend = None             # (edge, odd-neighbor) waiting for a partner
        for e, u in vadj[x]:
            if eused[e] or not res[u] or u == x:
                continue
            if pend is None:
                pend = (e, u)
                continue
            e0, u0 = pend
            if eused[e0] or not res[u0]:
                pend = (e, u)
                continue
            keep[e0] = keep[e] = False
            eused[e0] = eused[e] = True
            res[u0] = res[u] = False
            pend = None

    verts, edge_at, is_nand = _euler_stream(
        output_node_input_indices, nor_mask, keep)

    # Relabel rows by first occurrence so the gather walks HBM sequentially.
    first = np.full(N_ROWS, -1, dtype=np.int64)
    seen = np.zeros(N_ROWS, dtype=bool)
    rank = 0
    for v in verts:
        if not seen[v]:
            seen[v] = True
            first[v] = rank
            rank += 1
    first[~seen] = np.arange(rank, N_ROWS)
    sverts = first[verts]

    # Build the shared chunk plan: stream chunks with the classic chunks
    # interleaved every ~5 positions (none in the first 4: clean ramp).
    half_pairs = -(-(len(edge_at)) // OUT_SPLIT)
    sizes = _stream_sizes(half_pairs)
    cls = np.flatnonzero(~keep)
    cls_nand = cls[~mask[cls]]
    cls_nor = cls[mask[cls]]
    classic = []
    for op, rows_c in (('or', cls_nand), ('and', cls_nor)):
        cap_c = -(-max(1, -(-len(rows_c) // OUT_SPLIT)) // P) * P
        while cap_c > 1024:
            classic.append(('c', 1024, op))
            cap_c -= 1024
        classic.append(('c', cap_c, op))
    plan = [('s', n) for n in sizes]
    step = max(2, (len(plan) - 4) // max(1, len(classic)))
    pos = 4
    for entry in classic:
        pos = min(pos, len(plan))
        plan.insert(pos, entry)
        pos += step + 1

    # Pad stream arrays to the plan capacity.
    tp = OUT_SPLIT * sum((n // P - 1) * P for k, *r in plan if k == 's'
                         for n in [r[0]])
    assert len(edge_at) <= tp, (len(edge_at), tp)
    sverts = np.concatenate(
        [sverts, np.full(tp + 1 - len(sverts), sverts[-1], dtype=np.int64)])
    edge_at = np.concatenate(
        [edge_at, np.full(tp - len(edge_at), -1, dtype=np.int64)])
    is_nand = np.concatenate(
        [is_nand, np.zeros(tp - len(is_nand), dtype=bool)])

    lo = first[np.minimum(idx[:, 0], idx[:, 1])]
    hi = first[np.maximum(idx[:, 0], idx[:, 1])]

    idx_tabs, msk_tabs, row_orders = [], [], []
    for h in range(OUT_SPLIT):
        planes, mcols, ro = [], [], []
        cls_done = {}
        base = h * (tp // OUT_SPLIT)
        for entry in plan:
            if entry[0] == 's':
                n = entry[1]
                ncol = n // P
                dcol = ncol - 1
                j = np.arange(n)
                slots = base + (j % P) * dcol + j // P
                planes.append(_wrap_idxs(sverts[slots]))
                p_i = np.arange(P)[:, None]
                b_i = np.arange(dcol)[None, :]
                pairs = base + dcol * p_i + b_i
                e = edge_at[pairs]
                mcols.append(np.where(is_nand[pairs], -1, 0)
                             .astype(np.int32))
                ro.append(e.reshape(-1))
                base += dcol * P
            else:
                _, n_c, op = entry
                rows_c = cls_nand if op == 'or' else cls_nor
                hcnt = -(-len(rows_c) // OUT_SPLIT)
                sel = rows_c[h * hcnt:(h + 1) * hcnt]
                sel = sel[np.argsort(lo[sel], kind='stable')]
                done = cls_done.setdefault((h, op), 0)
                sel = sel[done:done + n_c]
                cls_done[(h, op)] = done + n_c
                ol = np.full(n_c, -1, dtype=np.int64)
                ol[:len(sel)] = sel
                la = np.where(ol >= 0, lo[np.maximum(ol, 0)], 0)
                lb = np.where(ol >= 0, hi[np.maximum(ol, 0)], 0)
                ncol = n_c // P
                # idx-list position j = (j%128 partition, j//128 block);
                # column (p, b) must hold output ol[p*ncol + b]
                j = np.arange(n_c)
                q = (j % P) * ncol + j // P
                planes.append(_wrap_idxs(la[q]))
                planes.append(_wrap_idxs(lb[q]))
                ro.append(ol)
        idx_tabs.append(np.concatenate(planes, axis=1))
        msk_tabs.append(np.concatenate(mcols, axis=1)[:, :, None])
        row_orders.append(np.concatenate(ro))
    return first, idx_tabs, msk_tabs, row_orders, plan


def _build(plan):
    """One SPMD program for all 8 cores, following the shared chunk plan."""
    idx_cols = sum((n // 16 if k == 's' else 2 * (n // 16))
                   for k, n, *r in plan)
    msk_cols = sum(n // P - 1 for k, n, *r in plan if k == 's')
    out_free = sum((n // P - 1 if k == 's' else n // P) * WS
                   for k, n, *r in plan)
    nc = bacc.Bacc("TRN2", target_bir_lowering=False, debug=False,
                   num_devices=N_CORES, num_swdge_queues=NUM_SWDGE_QUEUES)
    x = nc.dram_tensor("x", [N_ROWS, WS], mybir.dt.int32,
                       kind="ExternalInput").ap()
    ia = nc.dram_tensor("ia", [P, idx_cols], mybir.dt.int16,
                        kind="ExternalInput").ap()
    mk = nc.dram_tensor("mk", [P, msk_cols, 1], mybir.dt.int32,
                        kind="ExternalInput").ap()
    out = nc.dram_tensor("out", [P, out_free], mybir.dt.int32,
                         kind="ExternalOutput").ap()
    i16 = mybir.dt.int16
    AND = mybir.AluOpType.bitwise_and
    OR = mybir.AluOpType.bitwise_or

    with ExitStack() as ctx:
        tc = ctx.enter_context(tile.TileContext(nc))
        idxp = ctx.enter_context(tc.tile_pool(name="idx", bufs=1))
        datap = ctx.enter_context(tc.tile_pool(name="data", bufs=3))
        ta_i = idxp.tile([P, idx_cols], i16)
        msk = idxp.tile([P, msk_cols, 1], mybir.dt.int32)
        # Load the first chunk's indices first so its gather starts early.
        c0 = plan[0][1] // 16
        nc.sync.dma_start(ta_i[:, :c0], ia[:, :c0])
        nc.scalar.dma_start(msk[:, :, :], mk[:, :, :])
        nc.sync.dma_start(ta_i[:, c0:], ia[:, c0:])
        pending = []
        icol = 0
        ocol = 0
        mcol = 0
        for ci, entry in enumerate(plan):
            qa, qb = CHUNK_QUEUES[ci % 2], CHUNK_QUEUES[(ci + 1) % 2]
            if entry[0] == 's':
                n = entry[1]
                ncol = n // P
                dcol = ncol - 1
                isl = slice(icol, icol + n // 16)
                icol += n // 16
                t = datap.tile([P, 8, WS], mybir.dt.int32, tag="t", bufs=4)
                nc.gpsimd.dma_gather(
                    out_ap=t[:, :ncol, :], in_ap=x, idxs_ap=ta_i[:, isl],
                    num_idxs=n, num_idxs_reg=n,
                    elem_size=WS, queue_num=qa)
                if len(pending) >= WRITE_LAG:
                    o_ap, r_ap = pending.pop(0)
                    nc.gpsimd.dma_start(o_ap, r_ap, single_packet=True)
                r = datap.tile([P, 8, WS], mybir.dt.int32, tag="r", bufs=4)
                o = datap.tile([P, 7, WS], mybir.dt.int32, tag="o", bufs=2)
                in0 = t[:, :dcol, :].rearrange('p b w -> p (b w)')
                in1 = t[:, 1:ncol, :].rearrange('p b w -> p (b w)')
                nc.vector.tensor_tensor(
                    out=r[:, :dcol, :].rearrange('p b w -> p (b w)'),
                    in0=in0, in1=in1, op=AND)
                nc.vector.tensor_tensor(
                    out=o[:, :dcol, :].rearrange('p b w -> p (b w)'),
                    in0=in0, in1=in1, op=OR)
                # NAND columns take the OR result.
                ms = msk[:, mcol:mcol + dcol, :]
                mcol += dcol
                mb, _ = broadcast_tensor_aps(ms, r[:, :dcol, :])
                nc.vector.copy_predicated(r[:, :dcol, :], mb, o[:, :dcol, :])
                pending.append((
                    out[:, ocol:ocol + dcol * WS].rearrange(
                        'p (b w) -> p b w', b=dcol, w=WS),
                    r[:, :dcol, :]))
                ocol += dcol * WS
            else:
                _, n, op = entry
                ncol = n // P
                isl_a = slice(icol, icol + n // 16)
                isl_b = slice(icol + n // 16, icol + 2 * (n // 16))
                icol += 2 * (n // 16)
                t = datap.tile([P, 8, WS], mybir.dt.int32, tag="t", bufs=4)
                nc.gpsimd.dma_gather(
                    out_ap=t[:, :ncol, :], in_ap=x, idxs_ap=ta_i[:, isl_a],
                    num_idxs=n, num_idxs_reg=n,
                    elem_size=WS, queue_num=qa)
                if len(pending) >= WRITE_LAG:
                    o_ap, r_ap = pending.pop(0)
                    nc.gpsimd.dma_start(o_ap, r_ap, single_packet=True)
                tb = datap.tile([P, 8, WS], mybir.dt.int32, tag="tb", bufs=1)
                nc.gpsimd.dma_gather(
                    out_ap=tb[:, :ncol, :], in_ap=x, idxs_ap=ta_i[:, isl_b],
                    num_idxs=n, num_idxs_reg=n,
                    elem_size=WS, queue_num=qb)
                r = datap.tile([P, 8, WS], mybir.dt.int32, tag="r", bufs=4)
                alu = OR if op == 'or' else AND
                nc.vector.tensor_tensor(
                    out=r[:, :ncol, :].rearrange('p b w -> p (b w)'),
                    in0=t[:, :ncol, :].rearrange('p b w -> p (b w)'),
                    in1=tb[:, :ncol, :].rearrange('p b w -> p (b w)'),
                    op=alu)
                pending.append((
                    out[:, ocol:ocol + ncol * WS].rearrange(
                        'p (b w) -> p b w', b=ncol, w=WS),
                    r[:, :ncol, :]))
                ocol += ncol * WS
        for o_ap, r_ap in pending:
            nc.gpsimd.dma_start(o_ap, r_ap, single_packet=True)
    nc.finalize()
    return nc


def _in_maps(input_bitarrays, relabel, idx_tabs, msk_tabs):
    cx = ~np.asarray(input_bitarrays)   # complemented table (De Morgan)
    cxp = np.empty_like(cx)
    cxp[relabel] = cx                   # row r of cx lands at relabel[r]
    slices = [np.ascontiguousarray(cxp[:, w * WS:(w + 1) * WS])
              for w in range(WORD_SPLIT)]
    return [{"x": slices[c % WORD_SPLIT],
             "ia": idx_tabs[c // WORD_SPLIT],
             "mk": msk_tabs[c // WORD_SPLIT]} for c in range(N_CORES)]


def kernel(input_bitarrays, output_node_input_indices, nor_mask):
    x = np.asarray(input_bitarrays)
    assert x.shape == (N_ROWS, W_FULL) and x.dtype == np.int32
    relabel, idx_tabs, msk_tabs, row_orders, plan = _prepare(
        output_node_input_indices, nor_mask)
    nc = _build(plan)
    res = run_bass_kernel_spmd(nc, _in_maps(x, relabel, idx_tabs, msk_tabs),
                               core_ids=list(range(N_CORES)))

    result = np.empty((N_ROWS, W_FULL), dtype=np.int32)
    for c in range(N_CORES):
        h, w = c // WORD_SPLIT, c % WORD_SPLIT
        ro = row_orders[h]
        arr = res.results[c]["out"]     # [128, out_free]
        pieces = []
        off = 0
        for k, n, *rest in plan:
            dcol = n // P - (1 if k == 's' else 0)
            blk = arr[:, off:off + dcol * WS].reshape(P, dcol, WS)
            pieces.append(blk.reshape(P * dcol, WS))
            off += dcol * WS
        rows = np.concatenate(pieces, axis=0)
        valid = ro >= 0
        result[ro[valid], w * WS:(w + 1) * WS] = rows[valid]
    return result


# revision 28
# speedup vs baseline: 1.0572x; 1.0572x over previous
"""Bipartite NAND/NOR graph layer on 8 Trainium2 NeuronCores — Euler-stream
formulation.

Problem: out[i] = ~(x[a_i] & x[b_i]) if not nor_mask[i] else ~(x[a_i] | x[b_i])
with x: [32768, 2048] int32, (a, b): [32768, 2] indices, nor_mask: [32768] bool.

The baseline gathers TWO 2 KiB operand rows per output (3 rows of HBM traffic
per output incl. the write; ~96 MiB/core at the ~358 GB/s per-core HBM
roofline).  This kernel cuts traffic to ~2.7 rows/output and is paced by the
DVE select (see below):

- View outputs as edges of a multigraph over the 32768 input rows.  An Euler
  trail visits every edge once and consecutive trail edges SHARE a vertex, so
  gathering the trail's vertex sequence once gives both operands of every
  output: out[s] = f(stream[s], stream[s+1]).  Odd-degree vertices are paired
  with virtual edges (junk outputs the host discards); per-component trails
  are concatenated with one junk slot between components.  For this instance:
  32768 edges + 8052 junk slots -> ~1.25 gathered rows per output instead of 2.
- De Morgan on the complemented table cx = ~x: NAND = cx_a | cx_b,
  NOR = cx_a & cx_b.  The stream mixes both ops; the device computes AND and
  OR of each adjacent pair and resolves per-output with copy_predicated using
  a [128, dcol, 1] mask broadcast (stride 0) along the 512-word axis.  A
  2-instruction select is provably impossible with the available ALU ops, and
  bitwise TensorTensor is DVE-only (Pool rejects it), so the 3 DVE passes
  (~0.96 G col/s each) are the pacer: ~11.6 us per 1024-slot chunk vs ~11 us
  of DMA.  Measured ~306 us vs ~319 us for the 2-gather baseline in the same
  session.
- Chunk layout: a chunk gathers n slots as [128 partitions, ncol = n/128 slot
  columns]; idx order is permuted so slot (p, b) holds stream position
  base + (ncol-1)*p + b.  Each partition covers ncol-1 pairs; its last slot
  column duplicates the next partition's first slot (8/7 gather overhead), so
  pairs never cross partitions (compute engines cannot read partition-shifted
  views - BIR checkLegalPartitionAccess).  Small chunks at both ends of the
  schedule shorten pipeline ramp and drain.
- Rows are relabeled by first occurrence in the stream and the host uploads
  cx permuted accordingly, so the gather walks HBM nearly sequentially.
- Sharding: word axis split 4 ways (512-word = 2 KiB slices), stream split in
  2 halves -> 8 cores, zero cross-core traffic, one SPMD program (per-core
  data: x word-slice, half-stream index + mask tables).
- Ring layout per the baseline's findings: gathers alternate SWDGE rings 1/2,
  writes alone on ring 0 with single_packet=True, issued 2 chunks behind.
"""
import sys
sys.path.insert(0, "/opt/trn_rl_repo")

import numpy as np
from contextlib import ExitStack

import concourse.bass as bass
import concourse.tile as tile
from concourse import bacc, mybir
from concourse.bass import broadcast_tensor_aps
from concourse.bass_utils import run_bass_kernel_spmd

N_ROWS = 32768          # input rows == output rows
W_FULL = 2048           # int32 words per row
N_CORES = 8
WORD_SPLIT = 4
OUT_SPLIT = 2
WS = W_FULL // WORD_SPLIT   # 512 words per core slice (2 KiB)
P = 128
# The chunk PLAN is instance-derived in _prepare and shared by all 8 cores:
# a list of ('s', n) stream chunks (n gathered slots, (n/128-1)*128 pairs,
# 3-pass select) and ('c', n, op) classic chunks (n outputs, two gathers,
# single op pass).  Small stream chunks at the start shorten the ramp;
# classic chunks (DMA-heavy, DVE-light) are interleaved through the middle
# to balance engine load.
def _stream_sizes(pairs_needed):
    sizes = [512, 640]
    cap = 384 + 512
    while cap + 384 < pairs_needed:
        sizes.append(1024)
        cap += 896
    sizes.append(512)
    return sizes
CHUNK_QUEUES = (1, 2)   # gather ring by chunk parity
NUM_SWDGE_QUEUES = 3
WRITE_LAG = 2


def _euler_stream(output_node_input_indices, nor_mask, keep):
    """Build the Euler stream over the kept subset of the output multigraph.

    Returns (verts, edge_at, is_nand) where verts[s] is the input row gathered
    at stream slot s, edge_at[s] is the ORIGINAL output row computed from
    slots (s, s+1) (-1 for junk pairs), is_nand[s] its op.
    """
    mask = np.asarray(nor_mask).astype(bool)
    orig = np.flatnonzero(keep)
    idx = np.asarray(output_node_input_indices).astype(np.int64)[orig]
    m = idx.shape[0]
    n = N_ROWS

    deg = np.zeros(n, dtype=np.int64)
    np.add.at(deg, idx[:, 0], 1)
    np.add.at(deg, idx[:, 1], 1)

    # Components via union-find over real edges.
    parent = np.arange(n)

    def find(v):
        while parent[v] != v:
            parent[v] = parent[parent[v]]
            v = parent[v]
        return v

    for a, b in idx:
        ra, rb = find(a), find(b)
        if ra != rb:
            parent[ra] = rb
    root = np.array([find(v) for v in range(n)])

    comp_of = {}
    comps = []          # per component: list of member vertices with edges
    for v in range(n):
        if deg[v] == 0:
            continue
        r = root[v]
        c = comp_of.get(r)
        if c is None:
            c = comp_of[r] = len(comps)
            comps.append([])
        comps[c].append(v)

    # Virtual edges: within each component pair up odd vertices, leaving two
    # endpoints open (Euler path); fully even components get a circuit.
    starts = []
    extra = [[], []]    # endpoints of virtual edges
    for members in comps:
        odds = [v for v in members if deg[v] % 2 == 1]
        if odds:
            starts.append(odds[0])
            for i in range(1, len(odds) - 1, 2):
                extra[0].append(odds[i])
                extra[1].append(odds[i + 1])
        else:
            starts.append(members[0])

    # CSR adjacency over real + virtual edges.  Virtual edges have eid >= m.
    ea = np.concatenate([idx[:, 0], np.asarray(extra[0], dtype=np.int64)])
    eb = np.concatenate([idx[:, 1], np.asarray(extra[1], dtype=np.int64)])
    me = len(ea)
    half_v = np.concatenate([ea, eb])       # endpoint list, edge i at i, i+me
    half_o = np.concatenate([eb, ea])       # the other endpoint
    order = np.argsort(half_v, kind="stable")
    adj_other = half_o[order]
    adj_eid = (np.arange(2 * me) % me)[order]
    adj_start = np.zeros(n + 1, dtype=np.int64)
    np.add.at(adj_start, half_v + 1, 1)
    adj_start = np.cumsum(adj_start)

    used = np.zeros(me, dtype=bool)
    ptr = adj_start[:-1].copy()
    verts_l, edges_l = [], []
    for start in starts:
        # Iterative Hierholzer from `start`.
        stack_v = [start]
        stack_e = [-2]          # edge taken to REACH stack_v[i] (-2 for root)
        path_v, path_e = [], []
        while stack_v:
            v = stack_v[-1]
            pv = ptr[v]
            stop = adj_start[v + 1]
            while pv < stop and used[adj_eid[pv]]:
                pv += 1
            ptr[v] = pv
            if pv < stop:
                e = adj_eid[pv]
                used[e] = True
                ptr[v] = pv + 1
                stack_v.append(adj_other[pv])
                stack_e.append(e)
            else:
                path_v.append(stack_v.pop())
                path_e.append(stack_e.pop())
        path_v.reverse()
        path_e.reverse()
        # path_e[i+1] is the edge between path_v[i] and path_v[i+1].
        if verts_l:
            edges_l.append(-1)  # junk pair between components
        verts_l.extend(path_v)
        edges_l.extend(e if e < m else -1 for e in path_e[1:])

    verts = np.asarray(verts_l, dtype=np.int64)
    edge_at = np.asarray(edges_l, dtype=np.int64)
    assert len(verts) == len(edge_at) + 1
    assert np.count_nonzero(edge_at >= 0) == m
    real = edge_at >= 0
    edge_at[real] = orig[edge_at[real]]   # back to original output rows
    is_nand = np.zeros(len(edge_at), dtype=bool)
    is_nand[real] = ~mask[edge_at[real]]
    return verts, edge_at, is_nand


def _wrap_idxs(idx_chunk):
    """[n] int -> [128, n/16] int16 wrapped in 16 partitions, replicated
    across the 8 gpsimd core windows."""
    n = len(idx_chunk)
    assert n % 16 == 0
    blk = idx_chunk.astype(np.int16).reshape(n // 16, 16).T  # [16, n/16]
    return np.tile(blk, (8, 1))


def _prepare(output_node_input_indices, nor_mask):
    """Returns (relabel, idx_tabs, msk_tabs, row_orders, plan)."""
    idx = np.asarray(output_node_input_indices).astype(np.int64)
    mask = np.asarray(nor_mask).astype(bool)

    # Pull out a greedy matching of edges joining two odd-degree vertices:
    # removing such an edge kills one virtual (junk) trail pairing AND the
    # output it represents is computed in an op-pure classic 2-gather chunk
    # (1 DVE pass instead of 3).
    deg = np.zeros(N_ROWS, dtype=np.int64)
    np.add.at(deg, idx[:, 0], 1)
    np.add.at(deg, idx[:, 1], 1)
    odd = deg % 2 == 1
    taken = np.zeros(N_ROWS, dtype=bool)
    keep = np.ones(len(idx), dtype=bool)
    for e, (u, w) in enumerate(idx):
        if u != w and odd[u] and odd[w] and not taken[u] and not taken[w]:
            keep[e] = False
            taken[u] = taken[w] = True

    verts, edge_at, is_nand = _euler_stream(
        output_node_input_indices, nor_mask, keep)

    # Relabel rows by first occurrence so the gather walks HBM sequentially.
    first = np.full(N_ROWS, -1, dtype=np.int64)
    seen = np.zeros(N_ROWS, dtype=bool)
    rank = 0
    for v in verts:
        if not seen[v]:
            seen[v] = True
            first[v] = rank
            rank += 1
    first[~seen] = np.arange(rank, N_ROWS)
    sverts = first[verts]

    # Build the shared chunk plan: stream chunks with the classic chunks
    # interleaved every ~5 positions (none in the first 4: clean ramp).
    half_pairs = -(-(len(edge_at)) // OUT_SPLIT)
    sizes = _stream_sizes(half_pairs)
    cls = np.flatnonzero(~keep)
    cls_nand = cls[~mask[cls]]
    cls_nor = cls[mask[cls]]
    classic = []
    for op, rows_c in (('or', cls_nand), ('and', cls_nor)):
        cap_c = -(-max(1, -(-len(rows_c) // OUT_SPLIT)) // P) * P
        while cap_c > 1024:
            classic.append(('c', 1024, op))
            cap_c -= 1024
        classic.append(('c', cap_c, op))
    plan = [('s', n) for n in sizes]
    pos = 4
    for entry in classic:
        pos = min(pos, len(plan))
        plan.insert(pos, entry)
        pos += 5

    # Pad stream arrays to the plan capacity.
    tp = OUT_SPLIT * sum((n // P - 1) * P for k, *r in plan if k == 's'
                         for n in [r[0]])
    assert len(edge_at) <= tp, (len(edge_at), tp)
    sverts = np.concatenate(
        [sverts, np.full(tp + 1 - len(sverts), sverts[-1], dtype=np.int64)])
    edge_at = np.concatenate(
        [edge_at, np.full(tp - len(edge_at), -1, dtype=np.int64)])
    is_nand = np.concatenate(
        [is_nand, np.zeros(tp - len(is_nand), dtype=bool)])

    lo = first[np.minimum(idx[:, 0], idx[:, 1])]
    hi = first[np.maximum(idx[:, 0], idx[:, 1])]

    idx_tabs, msk_tabs, row_orders = [], [], []
    for h in range(OUT_SPLIT):
        planes, mcols, ro = [], [], []
        cls_done = {}
        base = h * (tp // OUT_SPLIT)
        for entry in plan:
            if entry[0] == 's':
                n = entry[1]
                ncol = n // P
                dcol = ncol - 1
                j = np.arange(n)
                slots = base + (j % P) * dcol + j // P
                planes.append(_wrap_idxs(sverts[slots]))
                p_i = np.arange(P)[:, None]
                b_i = np.arange(dcol)[None, :]
                pairs = base + dcol * p_i + b_i
                e = edge_at[pairs]
                mcols.append(np.where(is_nand[pairs], -1, 0)
                             .astype(np.int32))
                ro.append(e.reshape(-1))
                base += dcol * P
            else:
                _, n_c, op = entry
                rows_c = cls_nand if op == 'or' else cls_nor
                hcnt = -(-len(rows_c) // OUT_SPLIT)
                sel = rows_c[h * hcnt:(h + 1) * hcnt]
                sel = sel[np.argsort(lo[sel], kind='stable')]
                done = cls_done.setdefault((h, op), 0)
                sel = sel[done:done + n_c]
                cls_done[(h, op)] = done + n_c
                ol = np.full(n_c, -1, dtype=np.int64)
                ol[:len(sel)] = sel
                la = np.where(ol >= 0, lo[np.maximum(ol, 0)], 0)
                lb = np.where(ol >= 0, hi[np.maximum(ol, 0)], 0)
                ncol = n_c // P
                # idx-list position j = (j%128 partition, j//128 block);
                # column (p, b) must hold output ol[p*ncol + b]
                j = np.arange(n_c)
                q = (j % P) * ncol + j // P
                planes.append(_wrap_idxs(la[q]))
                planes.append(_wrap_idxs(lb[q]))
                ro.append(ol)
        idx_tabs.append(np.concatenate(planes, axis=1))
        msk_tabs.append(np.concatenate(mcols, axis=1)[:, :, None])
        row_orders.append(np.concatenate(ro))
    return first, idx_tabs, msk_tabs, row_orders, plan


def _build(plan):
    """One SPMD program for all 8 cores, following the shared chunk plan."""
    idx_cols = sum((n // 16 if k == 's' else 2 * (n // 16))
                   for k, n, *r in plan)
    msk_cols = sum(n // P - 1 for k, n, *r in plan if k == 's')
    out_free = sum((n // P - 1 if k == 's' else n // P) * WS
                   for k, n, *r in plan)
    nc = bacc.Bacc("TRN2", target_bir_lowering=False, debug=False,
                   num_devices=N_CORES, num_swdge_queues=NUM_SWDGE_QUEUES)
    x = nc.dram_tensor("x", [N_ROWS, WS], mybir.dt.int32,
                       kind="ExternalInput").ap()
    ia = nc.dram_tensor("ia", [P, idx_cols], mybir.dt.int16,
                        kind="ExternalInput").ap()
    mk = nc.dram_tensor("mk", [P, msk_cols, 1], mybir.dt.int32,
                        kind="ExternalInput").ap()
    out = nc.dram_tensor("out", [P, out_free], mybir.dt.int32,
                         kind="ExternalOutput").ap()
    i16 = mybir.dt.int16
    AND = mybir.AluOpType.bitwise_and
    OR = mybir.AluOpType.bitwise_or

    with ExitStack() as ctx:
        tc = ctx.enter_context(tile.TileContext(nc))
        idxp = ctx.enter_context(tc.tile_pool(name="idx", bufs=1))
        datap = ctx.enter_context(tc.tile_pool(name="data", bufs=3))
        ta_i = idxp.tile([P, idx_cols], i16)
        msk = idxp.tile([P, msk_cols, 1], mybir.dt.int32)
        # Load the first chunk's indices first so its gather starts early.
        c0 = plan[0][1] // 16
        nc.sync.dma_start(ta_i[:, :c0], ia[:, :c0])
        nc.scalar.dma_start(msk[:, :, :], mk[:, :, :])
        nc.sync.dma_start(ta_i[:, c0:], ia[:, c0:])
        pending = []
        icol = 0
        ocol = 0
        mcol = 0
        for ci, entry in enumerate(plan):
            qa, qb = CHUNK_QUEUES[ci % 2], CHUNK_QUEUES[(ci + 1) % 2]
            if entry[0] == 's':
                n = entry[1]
                ncol = n // P
                dcol = ncol - 1
                isl = slice(icol, icol + n // 16)
                icol += n // 16
                t = datap.tile([P, 8, WS], mybir.dt.int32, tag="t", bufs=4)
                nc.gpsimd.dma_gather(
                    out_ap=t[:, :ncol, :], in_ap=x, idxs_ap=ta_i[:, isl],
                    num_idxs=n, num_idxs_reg=n,
                    elem_size=WS, queue_num=qa)
                if len(pending) >= WRITE_LAG:
                    o_ap, r_ap = pending.pop(0)
                    nc.gpsimd.dma_start(o_ap, r_ap, single_packet=True)
                r = datap.tile([P, 8, WS], mybir.dt.int32, tag="r", bufs=4)
                o = datap.tile([P, 7, WS], mybir.dt.int32, tag="o", bufs=2)
                in0 = t[:, :dcol, :].rearrange('p b w -> p (b w)')
                in1 = t[:, 1:ncol, :].rearrange('p b w -> p (b w)')
                nc.vector.tensor_tensor(
                    out=r[:, :dcol, :].rearrange('p b w -> p (b w)'),
                    in0=in0, in1=in1, op=AND)
                nc.vector.tensor_tensor(
                    out=o[:, :dcol, :].rearrange('p b w -> p (b w)'),
                    in0=in0, in1=in1, op=OR)
                # NAND columns take the OR result.
                ms = msk[:, mcol:mcol + dcol, :]
                mcol += dcol
                mb, _ = broadcast_tensor_aps(ms, r[:, :dcol, :])
                nc.vector.copy_predicated(r[:, :dcol, :], mb, o[:, :dcol, :])
                pending.append((
                    out[:, ocol:ocol + dcol * WS].rearrange(
                        'p (b w) -> p b w', b=dcol, w=WS),
                    r[:, :dcol, :]))
                ocol += dcol * WS
            else:
                _, n, op = entry
                ncol = n // P
                isl_a = slice(icol, icol + n // 16)
                isl_b = slice(icol + n // 16, icol + 2 * (n // 16))
                icol += 2 * (n // 16)
                t = datap.tile([P, 8, WS], mybir.dt.int32, tag="t", bufs=4)
                nc.gpsimd.dma_gather(
                    out_ap=t[:, :ncol, :], in_ap=x, idxs_ap=ta_i[:, isl_a],
                    num_idxs=n, num_idxs_reg=n,
                    elem_size=WS, queue_num=qa)
                if len(pending) >= WRITE_LAG:
                    o_ap, r_ap = pending.pop(0)
                    nc.gpsimd.dma_start(o_ap, r_ap, single_packet=True)
                tb = datap.tile([P, 8, WS], mybir.dt.int32, tag="tb", bufs=1)
                nc.gpsimd.dma_gather(
                    out_ap=tb[:, :ncol, :], in_ap=x, idxs_ap=ta_i[:, isl_b],
                    num_idxs=n, num_idxs_reg=n,
                    elem_size=WS, queue_num=qb)
                r = datap.tile([P, 8, WS], mybir.dt.int32, tag="r", bufs=4)
                alu = OR if op == 'or' else AND
                nc.vector.tensor_tensor(
                    out=r[:, :ncol, :].rearrange('p b w -> p (b w)'),
                    in0=t[:, :ncol, :].rearrange('p b w -> p (b w)'),
                    in1=tb[:, :ncol, :].rearrange('p b w -> p (b w)'),
                    op=alu)
                pending.append((
                    out[:, ocol:ocol + ncol * WS].rearrange(
                        'p (b w) -> p b w', b=ncol, w=WS),
                    r[:, :ncol, :]))
                ocol += ncol * WS
        # Drain the last lagged writes on the idle HWDGE engines so they
        # overlap each other and ring 0's remaining work (the gather rings
        # are finished by now, so the packet-starvation hazard is gone).
        for i, (o_ap, r_ap) in enumerate(pending):
            (nc.sync if i % 2 == 0 else nc.scalar).dma_start(o_ap, r_ap)
    nc.finalize()
    return nc


def _in_maps(input_bitarrays, relabel, idx_tabs, msk_tabs):
    cx = ~np.asarray(input_bitarrays)   # complemented table (De Morgan)
    cxp = np.empty_like(cx)
    cxp[relabel] = cx                   # row r of cx lands at relabel[r]
    slices = [np.ascontiguousarray(cxp[:, w * WS:(w + 1) * WS])
              for w in range(WORD_SPLIT)]
    return [{"x": slices[c % WORD_SPLIT],
             "ia": idx_tabs[c // WORD_SPLIT],
             "mk": msk_tabs[c // WORD_SPLIT]} for c in range(N_CORES)]


def kernel(input_bitarrays, output_node_input_indices, nor_mask):
    x = np.asarray(input_bitarrays)
    assert x.shape == (N_ROWS, W_FULL) and x.dtype == np.int32
    relabel, idx_tabs, msk_tabs, row_orders, plan = _prepare(
        output_node_input_indices, nor_mask)
    nc = _build(plan)
    res = run_bass_kernel_spmd(nc, _in_maps(x, relabel, idx_tabs, msk_tabs),
                               core_ids=list(range(N_CORES)))

    result = np.empty((N_ROWS, W_FULL), dtype=np.int32)
    for c in range(N_CORES):
        h, w = c // WORD_SPLIT, c % WORD_SPLIT
        ro = row_orders[h]
        arr = res.results[c]["out"]     # [128, out_free]
        pieces = []
        off = 0
        for k, n, *rest in plan:
            dcol = n // P - (1 if k == 's' else 0)
            blk = arr[:, off:off + dcol * WS].reshape(P, dcol, WS)
            pieces.append(blk.reshape(P * dcol, WS))
            off += dcol * WS
        rows = np.concatenate(pieces, axis=0)
        valid = ro >= 0
        result[ro[valid], w * WS:(w + 1) * WS] = rows[valid]
    return result
